# revision 1
# baseline (speedup 1.0000x reference)
"""CropAndResize Trainium2 kernel.

Strategy (per 8-core SPMD):
- Boxes are packed into slots of 8 (one per core) with identical box_ind so the
  per-slot gather base address is static in the shared program.
- Per slot: dma_gather pulls the 2*28 needed image rows for all 64 channels
  (2 channels x 56 rows = 112 rows per "pair", 32 pairs) into SBUF with rows on
  partitions.
- Stage 1 (TensorE): A^T[x, (s,i)] = sum_rows G[row, x] * Wy[row, (s,i)] -- the
  y-interpolation fused with the transpose (x lands on partitions).
- Stage 2 (TensorE): out[(s,i), j] = sum_x A^T[x, (s,i)] * Wx[x, j] -- the
  x-interpolation. Extrapolation masking is folded into Wy/Wx as zero weights.
- Host computes the tiny index/weight tensors and unscrambles the raw output.
"""

import sys

sys.path.insert(0, "/opt/trn_rl_repo")

import numpy as np

import concourse.bass as bass
import concourse.bacc as bacc
import concourse.mybir as mybir
import concourse.tile as tile
from concourse.bass_utils import run_bass_kernel_spmd

B, C, H, W = 16, 64, 512, 512
NB = 512
CH = CW = 28
NCORES = 8
PAIRS = C // 2          # 32 channel pairs
NROW = 2 * CH           # 56 row slots per channel
KDIM = 2 * NROW         # 112 used partitions per pair


def _pack_slots(box_ind):
    """Group boxes by image id into slots of <=8 (one member per core)."""
    slots = []
    for b in range(B):
        ids = np.where(box_ind == b)[0]
        for i in range(0, len(ids), NCORES):
            slots.append((b, list(ids[i : i + NCORES])))
    if not slots:
        slots.append((0, []))
    return slots


def _host_prep(boxes, box_ind):
    """Per-box gather indices and interpolation weight matrices."""
    nb = boxes.shape[0]
    y1, x1, y2, x2 = (boxes[:, k].astype(np.float64) for k in range(4))
    i = np.arange(CH, dtype=np.float64)
    j = np.arange(CW, dtype=np.float64)

    h_scale = (y2 - y1) * (H - 1) / (CH - 1)
    w_scale = (x2 - x1) * (W - 1) / (CW - 1)
    in_y = y1[:, None] * (H - 1) + i[None, :] * h_scale[:, None]   # [nb, CH]
    in_x = x1[:, None] * (W - 1) + j[None, :] * w_scale[:, None]   # [nb, CW]

    y_ok = (in_y >= 0) & (in_y <= H - 1)
    x_ok = (in_x >= 0) & (in_x <= W - 1)
    y_top = np.clip(np.floor(in_y), 0, H - 1).astype(np.int64)
    y_bot = np.clip(np.ceil(in_y), 0, H - 1).astype(np.int64)
    x_lf = np.clip(np.floor(in_x), 0, W - 1).astype(np.int64)
    x_rt = np.clip(np.ceil(in_x), 0, W - 1).astype(np.int64)
    ly = (in_y - np.floor(in_y)) * y_ok
    lx = (in_x - np.floor(in_x)) * x_ok

    # Wy [nb, NROW, CH]: row slot 2i -> (1-ly_i), 2i+1 -> ly_i (masked)
    wy = np.zeros((nb, NROW, CH), dtype=np.float32)
    ii = np.arange(CH)
    wy[:, 2 * ii, ii] = ((1.0 - ly) * y_ok).astype(np.float32)
    wy[:, 2 * ii + 1, ii] = (ly * y_ok).astype(np.float32)

    # Wx [nb, W, CW]
    wx = np.zeros((nb, W, CW), dtype=np.float32)
    bi = np.arange(nb)[:, None].repeat(CW, 1)
    np.add.at(wx, (bi, x_lf, ii[None, :].repeat(nb, 0)),
              ((1.0 - lx) * x_ok).astype(np.float32))
    np.add.at(wx, (bi, x_rt, ii[None, :].repeat(nb, 0)),
              (lx * x_ok).astype(np.float32))

    # y row indices per slot r: r=2i -> y_top, r=2i+1 -> y_bot  [nb, NROW]
    yrows = np.zeros((nb, NROW), dtype=np.int64)
    yrows[:, 0::2] = y_top
    yrows[:, 1::2] = y_bot
    return wy, wx, yrows


def _build_program(slot_bs, nslot):
    nc = bacc.Bacc("TRN2", target_bir_lowering=False, debug=False)
    f32 = mybir.dt.float32
    img = nc.dram_tensor("img", [B * C * H, W], f32, kind="ExternalInput")
    idx = nc.dram_tensor("idx", [128, nslot * 256], mybir.dt.int16,
                         kind="ExternalInput")
    wy = nc.dram_tensor("wy", [KDIM, nslot * 56], f32, kind="ExternalInput")
    wx = nc.dram_tensor("wx", [128, nslot * 4 * CW], f32, kind="ExternalInput")
    outd = nc.dram_tensor("outd", [nslot, 56, PAIRS * CW], f32,
                          kind="ExternalOutput")

    with tile.TileContext(nc) as tc:
        with tc.tile_pool(name="singles", bufs=1) as singles, \
             tc.tile_pool(name="gpool", bufs=3) as gpool, \
             tc.tile_pool(name="asb", bufs=2) as asbp, \
             tc.tile_pool(name="ost", bufs=2) as ostp, \
             tc.tile_pool(name="apsum", bufs=6, space="PSUM") as apsum, \
             tc.tile_pool(name="opsum", bufs=2, space="PSUM") as opsum:
            idx_t = singles.tile([128, nslot * 256], mybir.dt.int16)
            nc.sync.dma_start(out=idx_t[:, :], in_=idx[:, :])
            wy_t = singles.tile([KDIM, nslot * 56], f32)
            nc.sync.dma_start(out=wy_t[:, :], in_=wy[:, :])
            wx_t = singles.tile([128, nslot * 4 * CW], f32)
            nc.sync.dma_start(out=wx_t[:, :], in_=wx[:, :])

            for k in range(nslot):
                b = slot_bs[k]
                src = img[b * C * H : (b + 1) * C * H, :]
                stg = ostp.tile([56, PAIRS * CW], f32)
                for half in range(2):
                    g = gpool.tile([128, 16, W], f32, tag="g")
                    nc.gpsimd.dma_gather(
                        out_ap=g[:, :, :],
                        in_ap=src,
                        idxs_ap=idx_t[:, (k * 2 + half) * 128 : (k * 2 + half + 1) * 128],
                        num_idxs=2048,
                        num_idxs_reg=2048,
                        elem_size=W,
                        single_packet=False,
                    )
                    o2 = opsum.tile([56, 16 * CW], f32, tag="o2")
                    for grp in range(2):
                        at = []
                        for _xc in range(4):
                            at_one = apsum.tile([128, 8 * 56], f32, tag="at")
                            at.append(at_one)
                        for pp in range(8):
                            hp = grp * 8 + pp
                            for xc in range(4):
                                nc.tensor.matmul(
                                    out=at[xc][:, pp * 56 : (pp + 1) * 56],
                                    lhsT=g[0:KDIM, hp, xc * 128 : (xc + 1) * 128],
                                    rhs=wy_t[0:KDIM, k * 56 : (k + 1) * 56],
                                    start=True, stop=True,
                                )
                        a_s = asbp.tile([128, 4, 8 * 56], f32, tag="as")
                        for xc in range(4):
                            nc.scalar.copy(out=a_s[:, xc, :], in_=at[xc][:, :])
                        for pp in range(8):
                            hp = grp * 8 + pp
                            for xc in range(4):
                                nc.tensor.matmul(
                                    out=o2[:, hp * CW : (hp + 1) * CW],
                                    lhsT=a_s[:, xc, pp * 56 : (pp + 1) * 56],
                                    rhs=wx_t[:, (k * 4 + xc) * CW : (k * 4 + xc + 1) * CW],
                                    start=(xc == 0), stop=(xc == 3),
                                )
                    nc.vector.tensor_copy(
                        out=stg[:, half * 16 * CW : (half + 1) * 16 * CW],
                        in_=o2[:, :])
                nc.sync.dma_start(out=outd[k, :, :], in_=stg[:, :])
    nc.compile()
    return nc


def _run(image, boxes, box_ind, trace=False, core_ids=None):
    image = np.ascontiguousarray(image, dtype=np.float32)
    boxes = np.asarray(boxes, dtype=np.float32)
    box_ind = np.asarray(box_ind, dtype=np.int32)
    nb = boxes.shape[0]

    slots = _pack_slots(box_ind)
    nslot = len(slots)
    wy_all, wx_all, yrows_all = _host_prep(boxes, box_ind)

    if core_ids is None:
        core_ids = list(range(NCORES))

    # per-core packed inputs
    img2d = image.reshape(B * C * H, W)
    in_maps = []
    ms = np.arange(128)
    rsl = ms % 56            # row slot within channel
    par = (ms >= 56).astype(np.int64)  # channel parity; pads in 112..127
    valid_m = ms < KDIM
    for c in range(max(core_ids) + 1):
        idx_np = np.zeros((128, nslot * 256), dtype=np.int16)
        wy_np = np.zeros((KDIM, nslot * 56), dtype=np.float32)
        wx_np = np.zeros((128, nslot * 4 * CW), dtype=np.float32)
        for k, (_, members) in enumerate(slots):
            if c >= len(members):
                continue
            bb = members[c]
            # indices: element kl = hp*128 + m (per half) -> value ch*512 + y
            yr = yrows_all[bb]            # [56]
            for half in range(2):
                vals = np.zeros((16, 128), dtype=np.int64)  # [hp, m]
                hps = np.arange(16)
                chan = 2 * (half * 16 + hps)[:, None] + par[None, :]
                yv = yr[rsl][None, :].repeat(16, 0)
                v = chan * 512 + yv
                v[:, ~valid_m] = 0
                kl = hps[:, None] * 128 + ms[None, :]       # 0..2047
                flat = np.zeros(2048, dtype=np.int64)
                flat[kl.ravel()] = v.ravel()
                blk = np.zeros((16, 128), dtype=np.int16)
                kk = np.arange(2048)
                blk[kk % 16, kk // 16] = flat
                col0 = (k * 2 + half) * 128
                for grp in range(8):
                    idx_np[16 * grp : 16 * grp + 16, col0 : col0 + 128] = blk
            wy_np[0:56, k * 56 : k * 56 + 28] = wy_all[bb]
            wy_np[56:112, k * 56 + 28 : k * 56 + 56] = wy_all[bb]
            for xc in range(4):
                wx_np[:, (k * 4 + xc) * CW : (k * 4 + xc + 1) * CW] = \
                    wx_all[bb][xc * 128 : (xc + 1) * 128, :]
        in_maps.append({"img": img2d, "idx": idx_np, "wy": wy_np, "wx": wx_np})

    slot_bs = [b for b, _ in slots]
    nc = _build_program(slot_bs, nslot)
    r = run_bass_kernel_spmd(nc, [in_maps[c] for c in core_ids],
                             core_ids=core_ids, trace=trace)
    try:
        r.nc = nc   # expose program for cost-model timing in test harness
    except Exception:
        pass

    out = np.zeros((nb, C, CH, CW), dtype=np.float32)
    for ci, c in enumerate(core_ids):
        raw = r.results[ci]["outd"]      # [nslot, 56, PAIRS*CW]
        for k, (_, members) in enumerate(slots):
            if c >= len(members):
                continue
            bb = members[c]
            t = raw[k].reshape(2, CH, PAIRS, CW)       # [s, i, p, j]
            out[bb] = t.transpose(2, 0, 1, 3).reshape(C, CH, CW)
    return out, r


def kernel(image, boxes, box_ind):
    out, _ = _run(image, boxes, box_ind)
    return out



# revision 3
# speedup vs baseline: 1.0180x; 1.0180x over previous
"""CropAndResize Trainium2 kernel, v2.

Design (8-core SPMD, boxes sharded over cores):
- Boxes are packed into slots of 8 (one per core) sharing box_ind and gather
  orientation; within a group, boxes are sorted by window width so each
  slot's static window size (nblk 128-px blocks) is tight.
- Image is converted to bf16 on host, stored in both [C,H,W] and transposed
  [C,W,H] layouts. Per box the orientation with the narrower window
  (x-extent vs y-extent) is gathered, halving average gather traffic.
- Per slot, 4 transpose-mode dma_gathers (one per 16-channel quarter; the
  int16 index space covers exactly one quarter at 256B stride) pull, for
  each (channel, tap) pair, only the nblk*128-px window of the box.
  Transpose mode lands the window axis on partitions.
- Stage A (TensorE, bf16): contract the window axis with per-box window
  weights: M1[(c2,t), u] per channel-pair tile, accumulated over nblk
  128-chunks.
- Stage B (TensorE, bf16): contract the 56 taps with a block-diagonal
  2-tap interpolation matrix: out[(c2,v), u].
- For X-orientation: taps are y (top/bottom rows), window axis is x,
  u=j, v=i. For Y-orientation: taps are x (left/right cols), window axis
  is y, u=i, v=j.
- Extrapolation masking is folded into the weights as zeros. Host computes
  the small per-box index/weight tensors and unscrambles the raw output.
"""

import sys

sys.path.insert(0, "/opt/trn_rl_repo")

import numpy as np
import ml_dtypes

B, C, H, W = 16, 64, 512, 512
NB = 512
CH = CW = 28
NCORES = 8
NQ = 4                  # channel quarters (16 ch each)
CQ = C // NQ            # 16
NT = 2 * CH             # 56 taps (28 floor + 28 ceil)
NIDX = CQ * NT          # 896 descs per quarter gather
ROWS128 = H * W // 128  # 128-elem blocks per channel (2048)
BPR = W // 128          # blocks per row/col (4)
BF16 = ml_dtypes.bfloat16


def _axis_prep(lo, hi):
    """For one axis: tap coords, fractions, validity, window blocks.

    lo/hi: [nb] normalized start/end. Returns per-box tap floor/ceil
    coords [nb,28], fraction [nb,28], ok [nb,28], blk0 [nb], nblk [nb].
    """
    n = lo.shape[0]
    i = np.arange(CH, dtype=np.float64)
    scale = (hi - lo) * (H - 1) / (CH - 1)
    pos = lo[:, None] * (H - 1) + i[None, :] * scale[:, None]
    ok = (pos >= 0) & (pos <= H - 1)
    fl = np.clip(np.floor(pos), 0, H - 1).astype(np.int64)
    ce = np.clip(np.ceil(pos), 0, H - 1).astype(np.int64)
    fr = (pos - np.floor(pos)) * ok
    mn = np.minimum(fl.min(1), ce.min(1))
    mx = np.maximum(fl.max(1), ce.max(1))
    blk0 = mn // 128
    nblk = (mx // 128) - blk0 + 1
    return fl, ce, fr, ok, blk0, nblk


def _slot_units(effs):
    """Sum over slots of 8 (sorted desc) of the slot max effective cost."""
    if len(effs) == 0:
        return 0
    v = np.sort(np.asarray(effs))[::-1]
    return sum(int(v[s]) for s in range(0, len(v), NCORES))


def _host_prep(boxes):
    """Per-axis tap/window data. ori=0 gathers an x-window (taps along y),
    ori=1 gathers a y-window (taps along x)."""
    y1, x1, y2, x2 = (boxes[:, k].astype(np.float64) for k in range(4))
    ax_y = _axis_prep(y1, y2)
    ax_x = _axis_prep(x1, x2)
    # effective DMA cost of a window: 256B descs (nblk=1) pay the <512B
    # latency doubling, so nblk=1 costs the same as nblk=2
    effx = np.maximum(ax_x[5], 2)
    effy = np.maximum(ax_y[5], 2)
    return {"y": ax_y, "x": ax_x, "effx": effx, "effy": effy}


def _pack_slots(box_ind, hp):
    """Choose per-box orientation and pack slots of <=8 boxes (one per
    core) sharing (image, orientation).

    Boxes with a strictly cheaper axis are forced to it; equal-cost boxes
    are flexible. Per image, the flexible split is brute-forced to minimize
    total slot units (sum over slots of the max member cost).
    """
    effx, effy = hp["effx"], hp["effy"]
    nblk_x, nblk_y = hp["x"][5], hp["y"][5]
    ori = np.zeros(len(box_ind), dtype=np.int64)
    slots = []
    for b in range(B):
        ids = np.where(box_ind == b)[0]
        if len(ids) == 0:
            continue
        fx = ids[effx[ids] < effy[ids]]
        fy = ids[effy[ids] < effx[ids]]
        fl = ids[effx[ids] == effy[ids]]
        fl = fl[np.argsort(-effx[fl], kind="stable")]
        best = None
        for nfx in range(len(fl) + 1):
            for big_to_x in (True, False):
                sel = fl[:nfx] if big_to_x else fl[len(fl) - nfx:]
                xm = np.concatenate([fx, sel])
                ym = np.concatenate([fy, np.setdiff1d(fl, sel,
                                                      assume_unique=True)])
                cost = _slot_units(effx[xm]) + _slot_units(effy[ym])
                if best is None or cost < best[0]:
                    best = (cost, xm, ym)
        _, xm, ym = best
        ori[ym] = 1
        for o, mem_ids, eff, nblk in ((0, xm, effx, nblk_x),
                                      (1, ym, effy, nblk_y)):
            if len(mem_ids) == 0:
                continue
            order = np.argsort(-(eff[mem_ids] * 8 + nblk[mem_ids]),
                               kind="stable")
            mem_ids = mem_ids[order]
            for s in range(0, len(mem_ids), NCORES):
                mem = list(mem_ids[s : s + NCORES])
                slots.append((b, o, mem, int(nblk[mem].max())))
    if not slots:
        slots.append((0, 0, [], 1))
    return slots, ori


def _finalize(hp, ori):
    """Per-box tap/window arrays for the chosen orientation."""
    (yfl, yce, yfr, yok, yb0, ynb) = hp["y"]
    (xfl, xce, xfr, xok, xb0, xnb) = hp["x"]
    nb = len(ori)
    taps = np.where(ori[:, None] == 0,
                    np.concatenate([yfl, yce], 1),
                    np.concatenate([xfl, xce], 1))
    tfr = np.where(ori[:, None] == 0, yfr, xfr)
    tok = np.where(ori[:, None] == 0, yok, xok)
    blk0 = np.where(ori == 0, xb0, yb0)
    nblk = np.where(ori == 0, xnb, ynb)
    ii = np.arange(CH)
    wwin = []
    for b in range(nb):
        if ori[b] == 0:
            fl, ce, fr, ok = xfl[b], xce[b], xfr[b], xok[b]
        else:
            fl, ce, fr, ok = yfl[b], yce[b], yfr[b], yok[b]
        wmat = np.zeros((int(nblk[b]) * 128, CH), dtype=np.float64)
        base = blk0[b] * 128
        np.add.at(wmat, (fl - base, ii), (1.0 - fr) * ok)
        np.add.at(wmat, (ce - base, ii), fr * ok)
        wwin.append(wmat)
    return {"ori": ori, "taps": taps, "tfr": tfr, "tok": tok,
            "blk0": blk0, "nblk": nblk, "wwin": wwin}


def _per_core_inputs(core, slots, hp):
    """Build this core's idx / wt / ww arrays for the shared program."""
    nslot = len(slots)
    sum_nblk = sum(s[3] for s in slots)
    idx_np = np.zeros((128, nslot * (NIDX // 16)), dtype=np.int16)
    wt_np = np.zeros((112, nslot * 56), dtype=BF16)   # block-diag tap side
    ww_np = np.zeros((128, sum_nblk * CW), dtype=BF16)  # window side

    ii = np.arange(CH)
    kk = np.arange(NIDX)
    c16 = np.arange(CQ)
    ww_col = 0
    for k, (_, _, members, snblk) in enumerate(slots):
        if core < len(members):
            bb = members[core]
            # gather indices: per quarter q, desc d = c16*56 + t
            tp = hp["taps"][bb]                   # [56] tap coords
            x0b = min(int(hp["blk0"][bb]), BPR - snblk)
            vals = (c16[:, None] * ROWS128 + tp[None, :] * BPR
                    + x0b).astype(np.int64)       # [16, 56]
            flat = vals.reshape(-1)
            blk = np.zeros((16, NIDX // 16), dtype=np.int16)
            blk[kk % 16, kk // 16] = flat
            col0 = k * (NIDX // 16)
            for grp in range(8):
                idx_np[16 * grp : 16 * grp + 16,
                       col0 : col0 + NIDX // 16] = blk
            # tap-side block-diag [112, 56]: rows (c2, t), cols (c2, v)
            fr, ok = hp["tfr"][bb], hp["tok"][bb]
            wt = np.zeros((NT, CH), dtype=np.float64)
            wt[ii, ii] = (1.0 - fr) * ok
            wt[CH + ii, ii] = fr * ok
            wtb = np.zeros((112, 56), dtype=np.float64)
            wtb[:56, :28] = wt
            wtb[56:, 28:] = wt
            wt_np[:, k * 56 : (k + 1) * 56] = wtb.astype(BF16)
            # window chunks: [128, snblk*28]; box window sits at chunk
            # offset blk0-x0b (>0 only when the clamp shifted it left)
            www = np.zeros((128, snblk * CW), dtype=np.float64)
            wb = hp["wwin"][bb]
            sh = int(hp["blk0"][bb]) - x0b
            for s in range(wb.shape[0] // 128):
                www[:, (s + sh) * CW : (s + sh + 1) * CW] = \
                    wb[s * 128 : (s + 1) * 128, :]
            ww_np[:, ww_col : ww_col + snblk * CW] = www.astype(BF16)
        ww_col += snblk * CW
    return {"idx": idx_np, "wy": wt_np, "wx": ww_np}


def _emulate_core(img_bf, imgT_bf, slots, core, inp):
    """Numpy replay of the device program for one core (bf16 rounding
    where the hardware would round)."""
    nslot = len(slots)
    flats = (img_bf.reshape(-1), imgT_bf.reshape(-1))
    out_raw = np.zeros((nslot, 56, 32 * CW), dtype=BF16)
    kk = np.arange(NIDX)
    ww_col = 0
    for k, (b, o, members, snblk) in enumerate(slots):
        img_flat = flats[o]
        g = np.zeros((128, snblk, NQ * NIDX), dtype=np.float32)
        span = np.arange(snblk * 128)
        for q in range(NQ):
            col0 = k * (NIDX // 16)
            blk = inp["idx"][0:16, col0 : col0 + NIDX // 16]
            flat = blk[kk % 16, kk // 16].astype(np.int64)
            base = (b * C + q * CQ) * H * W
            seg = img_flat[base + flat[:, None] * 128 + span[None, :]]
            g[:, :, q * NIDX : (q + 1) * NIDX] = (
                seg.astype(np.float32).reshape(NIDX, snblk, 128)
                .transpose(2, 1, 0))
        wt = inp["wy"][:, k * 56 : (k + 1) * 56].astype(np.float32)
        ww = inp["wx"][:, ww_col : ww_col + snblk * CW].astype(np.float32)
        a_s = np.zeros((128, 32 * CW), dtype=np.float32)
        for m in range(32):
            acc = np.zeros((112, CW), dtype=np.float32)
            for s in range(snblk):
                lhsT = g[:, s, m * 112 : (m + 1) * 112]
                acc += lhsT.T @ ww[:, s * CW : (s + 1) * CW]
            a_s[0:112, m * CW : (m + 1) * CW] = acc
        a_s = a_s.astype(BF16).astype(np.float32)    # PSUM->SBUF bf16 cast
        for m in range(32):
            out_raw[k, :, m * CW : (m + 1) * CW] = (
                wt[0:112, :].T @ a_s[0:112, m * CW : (m + 1) * CW]
            ).astype(BF16)
        ww_col += snblk * CW
    return out_raw


def _unscramble(raw, slots, core, out):
    for k, (_, o, members, _) in enumerate(slots):
        if core >= len(members):
            continue
        bb = members[core]
        t = raw[k].astype(np.float32).reshape(2, CH, 32, CW)  # [c2, v, m, u]
        full = t.transpose(2, 0, 1, 3).reshape(C, CH, CW)     # [c, v, u]
        out[bb] = full if o == 0 else full.transpose(0, 2, 1)


# ---------------------------------------------------------------------------
# Bass program
# ---------------------------------------------------------------------------

def _build_program(slot_bs, slot_ori, slot_nblk):
    import concourse.bass as bass
    import concourse.bacc as bacc
    import concourse.mybir as mybir
    import concourse.tile as tile

    nslot = len(slot_bs)
    sum_nblk = sum(slot_nblk)
    nc = bacc.Bacc("TRN2", target_bir_lowering=False, debug=False)
    f32 = mybir.dt.float32
    bf16 = mybir.dt.bfloat16
    # flat + tail pad: the last quarter's final overlapping-window
    # descriptor reads up to 384 elements past the image end
    img = nc.dram_tensor("img", [B * C * H * W + 512], bf16,
                         kind="ExternalInput")
    imgT = nc.dram_tensor("imgT", [B * C * H * W + 512], bf16,
                          kind="ExternalInput")
    idx = nc.dram_tensor("idx", [128, nslot * 56], mybir.dt.int16,
                         kind="ExternalInput")
    wy = nc.dram_tensor("wy", [112, nslot * 56], bf16, kind="ExternalInput")
    wx = nc.dram_tensor("wx", [128, sum_nblk * CW], bf16,
                        kind="ExternalInput")
    outd = nc.dram_tensor("outd", [nslot, 56, 32 * CW], bf16,
                          kind="ExternalOutput")

    with tile.TileContext(nc) as tc:
        with tc.tile_pool(name="singles", bufs=1) as singles, \
             tc.tile_pool(name="gpool", bufs=2) as gpool, \
             tc.tile_pool(name="asb", bufs=2) as asbp, \
             tc.tile_pool(name="ost", bufs=2) as ostp, \
             tc.tile_pool(name="apsum", bufs=2, space="PSUM") as apsum, \
             tc.tile_pool(name="opsum", bufs=2, space="PSUM") as opsum:
            idx_t = singles.tile([128, nslot * 56], mybir.dt.int16)
            nc.sync.dma_start(out=idx_t[:, :], in_=idx[:, :])
            wy_t = singles.tile([112, nslot * 56], bf16)
            nc.sync.dma_start(out=wy_t[:, :], in_=wy[:, :])
            wx_t = singles.tile([128, sum_nblk * CW], bf16)
            nc.sync.dma_start(out=wx_t[:, :], in_=wx[:, :])

            wx_col = 0
            for k in range(nslot):
                b = slot_bs[k]
                snblk = slot_nblk[k]
                src = imgT if slot_ori[k] else img
                g = [gpool.tile([128, 4, NIDX], bf16, tag=f"g{q}",
                                name=f"g{q}")
                     for q in range(NQ)]
                for q in range(NQ):
                    in_ap = bass.AP(
                        src, (b * C + q * CQ) * H * W,
                        [[128, CQ * ROWS128], [1, snblk * 128]])
                    nc.gpsimd.dma_gather(
                        out_ap=g[q][:, 0:snblk, :],
                        in_ap=in_ap,
                        idxs_ap=idx_t[:, k * 56 : (k + 1) * 56],
                        num_idxs=NIDX,
                        num_idxs_reg=NIDX,
                        elem_size=snblk * 128,
                        elem_step=128,
                        transpose=True,
                        single_packet=False,
                    )
                at = [apsum.tile([128, 16 * CW], f32, tag=f"at{h}",
                                 name=f"at{h}")
                      for h in range(2)]
                for m in range(32):
                    for s in range(snblk):
                        nc.tensor.matmul(
                            out=at[m // 16][0:112,
                                            (m % 16) * CW : (m % 16 + 1) * CW],
                            lhsT=g[m // 8][:, s,
                                           (m % 8) * 112 : (m % 8 + 1) * 112],
                            rhs=wx_t[:, (wx_col + s) * CW
                                     : (wx_col + s + 1) * CW],
                            start=(s == 0), stop=(s == snblk - 1),
                        )
                a_s = asbp.tile([128, 32 * CW], bf16, tag="as")
                for h in range(2):
                    nc.scalar.copy(out=a_s[:, h * 16 * CW : (h + 1) * 16 * CW],
                                   in_=at[h][:, :])
                ot = [opsum.tile([56, 16 * CW], f32, tag=f"ot{h}",
                                 name=f"ot{h}")
                      for h in range(2)]
                for h in range(2):
                    nc.tensor.matmul(
                        out=ot[h][:, :],
                        lhsT=wy_t[:, k * 56 : (k + 1) * 56],
                        rhs=a_s[0:112, h * 16 * CW : (h + 1) * 16 * CW],
                        start=True, stop=True,
                    )
                stg = ostp.tile([56, 32 * CW], bf16, tag="stg")
                nc.vector.tensor_copy(out=stg[:, 0 : 16 * CW], in_=ot[0][:, :])
                nc.scalar.copy(out=stg[:, 16 * CW : 32 * CW], in_=ot[1][:, :])
                nc.sync.dma_start(out=outd[k, :, :], in_=stg[:, :])
                wx_col += snblk
    nc.compile()
    return nc


def _run(image, boxes, box_ind, trace=False, core_ids=None):
    from concourse.bass_utils import run_bass_kernel_spmd

    image = np.ascontiguousarray(image, dtype=np.float32)
    boxes = np.asarray(boxes, dtype=np.float32)
    box_ind = np.asarray(box_ind, dtype=np.int32)
    nb = boxes.shape[0]

    pad = np.zeros(512, dtype=BF16)
    img_bf = np.concatenate([image.reshape(-1).astype(BF16), pad])
    imgT_bf = np.concatenate([
        np.ascontiguousarray(
            image.reshape(B * C, H, W).transpose(0, 2, 1)
        ).reshape(-1).astype(BF16), pad])
    hp0 = _host_prep(boxes)
    slots, ori = _pack_slots(box_ind, hp0)
    hp = _finalize(hp0, ori)

    if core_ids is None:
        core_ids = list(range(NCORES))

    in_maps = []
    for c in range(max(core_ids) + 1):
        m = _per_core_inputs(c, slots, hp)
        m["img"] = img_bf
        m["imgT"] = imgT_bf
        in_maps.append(m)

    nc = _build_program([s[0] for s in slots], [s[1] for s in slots],
                        [s[3] for s in slots])
    r = run_bass_kernel_spmd(nc, [in_maps[c] for c in core_ids],
                             core_ids=core_ids, trace=trace)
    try:
        r.nc = nc
    except Exception:
        pass

    out = np.zeros((nb, C, CH, CW), dtype=np.float32)
    for ci, c in enumerate(core_ids):
        _unscramble(r.results[ci]["outd"], slots, c, out)
    return out, r


def kernel(image, boxes, box_ind):
    out, _ = _run(image, boxes, box_ind)
    return out
